# revision 13
# baseline (speedup 1.0000x reference)
"""Causal self-attention (B=1, T=4096, E=1024, H=16, D=64) on 8 TRN2 NeuronCores.

v2 of the head-parallel kernel. Same matmul structure as the baseline
(feature-major flash attention, 2 heads/core, row-tiled S^T pairs, ones-column
denominator trick), but the softmax exp — the baseline's pacer at ~150us of
ACT busy — is split across ACT and DVE:

  - ACT: exact exp (activation, scale=0.125) for ~43% of blocks
  - DVE: exp2 bit-trick for the rest: bits_i16 = S*23.083 + 16249, bitcast
    bf16 (Schraudolph). ~1.8% RMS multiplicative error, mean-centered; the
    denominator (ones column) uses the same approximated P, so the softmax
    ratio cancels most of it. Measured end-to-end rel err stays ~1e-2.

GpSimd (Pool) cannot touch PSUM on TRN2, so it only gets the SBUF-side work:
causal masks (affine_select on P) and the denominator-reciprocal broadcast.
ACT also absorbs the QKV and out-projection PSUM evacuations (Identity/Copy
live in the same activation table as Exp — no table reloads).

Other deltas vs baseline:
  - normalize: reciprocal of the denominator row straight out of PSUM,
    broadcast of the reciprocal, one mul -> UnT (posb/drow copies dropped)
  - v-bias never enters the kernel: host adds (b_v @ w_out) to the output
  - O' emission lags exp by 2 blocks so DVE-path exps never stall the PE
  - x chunk 0 DMA'd per-e-chunk so the first QKV matmul starts ~1.5us earlier
  - V' transposes batched 4-per-PSUM-tile, one evac copy per step
  - warmup matmuls trimmed
"""

import sys

for _p in ("/opt/trn_rl_repo",):
    if _p not in sys.path:
        sys.path.insert(0, _p)

import ml_dtypes
import numpy as np

import concourse.bass as bass  # noqa: F401
import concourse.mybir as mybir
import concourse.tile as tile
from concourse import bacc
from concourse.bass_utils import run_bass_kernel_spmd
from concourse.masks import make_identity

T, E = 4096, 1024
H, D = 16, 64
NCORES = 8
HPC = H // NCORES          # heads per core = 2
HD = HPC * D               # hidden dims per core = 128
NT = T // 512              # 8 tq blocks of 512
NE = E // 128              # 8 e-chunks of 128
NTB = T // 128             # 32 tk blocks of 128

F32 = mybir.dt.float32
BF16 = mybir.dt.bfloat16
I16 = mybir.dt.int16
NPBF16 = np.dtype(ml_dtypes.bfloat16)
AF = mybir.ActivationFunctionType
ALU = mybir.AluOpType

# exp2 bit trick: exp(0.125*S) = 2^(0.125*log2(e)*S);
# bf16 bits = x*128 + 127*128, -7 centers the linear-interp error
EXP_A = 128.0 * 0.125 * 1.4426950408889634
EXP_B = 16256.0 - 7.0

# exp engine schedule (A=ACT exact, D=DVE bit trick). Early q-blocks have
# plenty of PE filler work (QKV/outproj) stretching the block period, so ACT
# can take a bigger share; late q-blocks run close to the raw S/O cadence and
# need the two engines balanced by their speed ratio.
FULL_PAT_EARLY = "ADDADAD"
FULL_PAT_LATE = "AADAD"
DIAG_PAT = "AD"
# out-projection tiles per pipeline step (skewed late: early steps already
# have QKV+vtrans fillers; late steps otherwise run out of PE filler and
# stall on exp)
OPROJ_QUOTA = {2: 2, 3: 4, 4: 4, 5: 6, 6: 8, 7: 4}


def _build_kernel():
    nc = bacc.Bacc("TRN2", target_bir_lowering=False, debug=False)

    xT = nc.dram_tensor("xT", [E, T], BF16, kind="ExternalInput")
    wq = nc.dram_tensor("wq", [E, HD], BF16, kind="ExternalInput")
    wk = nc.dram_tensor("wk", [E, HD], BF16, kind="ExternalInput")
    wv = nc.dram_tensor("wv", [E, HD], BF16, kind="ExternalInput")
    bqk = nc.dram_tensor("bqk", [2, HD, 1], F32, kind="ExternalInput")
    wo = nc.dram_tensor("wo", [HD, E], BF16, kind="ExternalInput")
    out = nc.dram_tensor("out", [T, E], BF16, kind="ExternalOutput")

    with tile.TileContext(nc) as tc:
        _body(nc, tc, xT, wq, wk, wv, bqk, wo, out)
    nc.compile()
    return nc


def _body(nc, tc, xT, wq, wk, wv, bqk, wo, out):
    from contextlib import ExitStack

    ctx = ExitStack()
    with ctx:
        const = ctx.enter_context(tc.tile_pool(name="const", bufs=1))
        big = ctx.enter_context(tc.tile_pool(name="big", bufs=1))
        xpool = ctx.enter_context(tc.tile_pool(name="xp", bufs=3))
        ppool = ctx.enter_context(tc.tile_pool(name="pp", bufs=4))
        opool = ctx.enter_context(tc.tile_pool(name="op", bufs=3))
        small = ctx.enter_context(tc.tile_pool(name="sm", bufs=4))
        ps_mm = ctx.enter_context(tc.tile_pool(name="ps_mm", bufs=2, space="PSUM"))
        ps_o = ctx.enter_context(tc.tile_pool(name="ps_o", bufs=2, space="PSUM"))
        ps_q = ctx.enter_context(tc.tile_pool(name="ps_q", bufs=2, space="PSUM"))

        # ---- constants / weights ----
        warm_src = const.tile([128, 512], BF16)
        nc.vector.memset(warm_src[:], 0.0)
        identb = const.tile([128, 128], BF16)
        make_identity(nc, identb[:])

        xs_map = {}

        def load_x(tcc, interleave=None):
            ts512 = slice(tcc * 512, (tcc + 1) * 512)
            xsb = xpool.tile([128, NE, 512], BF16, tag="xsb")
            if interleave is not None:
                # per-e-chunk DMAs so the first QKV matmuls start after
                # ~1/8 of the chunk has landed; weight DMAs interleaved
                for ec in range(NE):
                    nc.sync.dma_start(
                        xsb[:, ec, :], xT[ec * 128:(ec + 1) * 128, ts512]
                    )
                    for fn in dict(interleave).get(ec, ()):
                        fn()
            else:
                nc.sync.dma_start(
                    xsb[:], xT[:, ts512].rearrange("(a p) t -> p a t", p=128)
                )
            xs_map[tcc] = xsb

        wq_sb = const.tile([128, NE, HD], BF16)
        wk_sb = const.tile([128, NE, HD], BF16)
        wv_sb = const.tile([128, NE, HD], BF16)
        bq_sb = const.tile([128, 1], F32)
        bk_sb = const.tile([128, 1], F32)

        def _dma_w(w_sb_, w_dram_):
            return lambda: nc.sync.dma_start(
                w_sb_[:], w_dram_[:].rearrange("(a p) c -> p a c", p=128)
            )

        load_x(0, interleave={
            0: (_dma_w(wq_sb, wq),),
            1: (lambda: nc.sync.dma_start(bq_sb[:], bqk[0]),
                lambda: nc.sync.dma_start(bk_sb[:], bqk[1])),
            3: (_dma_w(wk_sb, wk),),
        })
        load_x(1, interleave={3: (_dma_w(wv_sb, wv),)})
        wo_sb = const.tile([128, E], BF16)
        nc.sync.dma_start(wo_sb[:], wo[:])

        qT = big.tile([128, T], BF16)
        kT = big.tile([128, T], BF16)
        vT = big.tile([128, T], BF16)
        V2 = big.tile([128, NTB, HPC, D + 1], BF16)
        UnT = big.tile([128, T], BF16)

        nc.gpsimd.memset(V2[:, :, :, D], 1.0)

        wparams = ((wq_sb, bq_sb), (wk_sb, bk_sb), (wv_sb, None))
        qkv_dst = (qT, kT, vT)

        def emit_qkv(tcc, m):
            w_sb, b_sb = wparams[m]
            ts512 = slice(tcc * 512, (tcc + 1) * 512)
            ps = ps_q.tile([128, 512], F32, tag="q")
            for ec in range(NE):
                nc.tensor.matmul(
                    ps[:], w_sb[:, ec, :], xs_map[tcc][:, ec, :],
                    start=(ec == 0), stop=(ec == NE - 1),
                )
            # evac on ACT (same act table as Exp): q/k get the bias,
            # v's bias is folded host-side
            if b_sb is not None:
                nc.scalar.activation(
                    qkv_dst[m][:, ts512], ps[:], AF.Identity,
                    bias=b_sb[:], scale=1.0,
                )
            else:
                nc.scalar.copy(qkv_dst[m][:, ts512], ps[:])

        def emit_vtrans(tcc):
            # all 4 tk blocks of the chunk into one PSUM tile, one evac.
            # start only on j==0: a start=True matmul pending-zeroes its whole
            # 2KB PSUM bank (ZERO_REGION_SIZE), which would wipe earlier
            # transposes sharing the tile.
            pst = ps_q.tile([128, 4, 128], BF16, tag="q")
            for j in range(4):
                tb = 4 * tcc + j
                nc.tensor.matmul(
                    pst[:, j, :], vT[:, tb * 128:(tb + 1) * 128], identb[:],
                    is_transpose=True, start=(j == 0), stop=(j == 3),
                )
            nc.vector.tensor_copy(
                V2[:, 4 * tcc:4 * tcc + 4, :, 0:D],
                pst[:].rearrange("p j (h d) -> p j h d", h=HPC),
            )

        def emit_piece(piece):
            kind = piece[0]
            if kind == "qkv":
                emit_qkv(piece[1], piece[2])
            elif kind == "vtrans":
                emit_vtrans(piece[1])
            else:
                _outproj_tile(nc, ps_q, opool, UnT, wo_sb, out, piece[1])

        def emit_S(qb, tb):
            f0 = max(0, tb * 128 - qb * 512)
            psS = ps_mm.tile([128, HPC, 512], F32, tag="mm")
            t0 = qb * 512 + f0
            t1 = (qb + 1) * 512
            for h in range(HPC):
                nc.tensor.matmul(
                    psS[:, h, f0:512],
                    kT[h * D:(h + 1) * D, tb * 128:(tb + 1) * 128],
                    qT[h * D:(h + 1) * D, t0:t1],
                    start=True, stop=True,
                )
            return psS

        exp_ct = [0, 0]  # full counter, diag counter

        def emit_exp(qb, tb, psS):
            """exp(0.125*S) -> P bf16; engine from schedule; returns P."""
            diag = tb >= 4 * qb
            f0 = max(0, tb * 128 - qb * 512)
            P = ppool.tile([128, HPC, 512], BF16, tag="P")
            if diag:
                eng = DIAG_PAT[exp_ct[1] % len(DIAG_PAT)]
                exp_ct[1] += 1
            else:
                pat = FULL_PAT_EARLY if qb < 5 else FULL_PAT_LATE
                eng = pat[exp_ct[0] % len(pat)]
                exp_ct[0] += 1
            if eng == "A":
                if f0 == 0:
                    nc.scalar.activation(
                        P[:].rearrange("p h f -> p (h f)"),
                        psS[:].rearrange("p h f -> p (h f)"),
                        AF.Exp, scale=0.125,
                    )
                else:
                    nc.scalar.activation(
                        P[:, :, f0:512], psS[:, :, f0:512], AF.Exp, scale=0.125
                    )
            else:
                if f0 == 0:
                    nc.vector.tensor_scalar(
                        P[:].rearrange("p h f -> p (h f)").bitcast(I16),
                        psS[:].rearrange("p h f -> p (h f)"),
                        EXP_A, EXP_B, ALU.mult, ALU.add,
                    )
                else:
                    nc.vector.tensor_scalar(
                        P[:, :, f0:512].bitcast(I16),
                        psS[:, :, f0:512],
                        EXP_A, EXP_B, ALU.mult, ALU.add,
                    )
            if diag:
                # causal mask on Pool: keep where tq >= tk
                nc.gpsimd.affine_select(
                    out=P[:, :, f0:512], in_=P[:, :, f0:512],
                    compare_op=mybir.AluOpType.is_ge,
                    fill=0.0,
                    base=qb * 512 + f0 - tb * 128,
                    channel_multiplier=-1,
                    pattern=[[0, HPC], [1, 512 - f0]],
                )
            return P

        # ---- prologue: short warmups span the x0/wq DMA wait and ramp
        # the PE clock; then chunk-0 q/k projections gate the first S ----
        for i in range(12):
            wps = ps_q.tile([128, 128], F32, tag="q")
            nc.tensor.matmul(wps[:], warm_src[:, 0:128], warm_src[:, 0:128],
                             start=True, stop=True)
        emit_qkv(0, 0)
        emit_qkv(0, 1)

        # ---- merged pipeline ----
        Stiles = {}
        oproj_next = [0]
        for step in range(NT):
            if step + 2 < NT:
                load_x(step + 2)
            pieces = []
            if step == 0:
                pieces += [("qkv", 0, 2)]
                pieces += [("vtrans", 0)]
            if step + 1 < NT:
                pieces += [("qkv", step + 1, m) for m in range(3)]
                pieces += [("vtrans", step + 1)]
            quota = OPROJ_QUOTA.get(step, 0)
            late = []
            if step == NT - 1:
                late += [("out", oproj_next[0] + i) for i in range(quota)]
            else:
                pieces += [("out", oproj_next[0] + i) for i in range(quota)]
            oproj_next[0] += quota
            qb = step
            nblk = 4 * (qb + 1)
            emit_at = {}
            for i, piece in enumerate(pieces):
                emit_at.setdefault((i + 1) * nblk // (len(pieces) + 1),
                                   []).append(piece)
            lo = nblk // 2
            for i, piece in enumerate(late):
                emit_at.setdefault(lo + (i + 1) * (nblk - lo) // (len(late) + 1),
                                   []).append(piece)
            pos = []
            for h in range(HPC):
                po = ps_o.tile([D + 1, 512], F32, tag="o")
                pos.append(po)

            def emit_O(tb, P):
                f0 = max(0, tb * 128 - qb * 512)
                for h in range(HPC):
                    nc.tensor.matmul(
                        pos[h][:, f0:512],
                        V2[:, tb, h, :],
                        P[:, h, f0:512],
                        start=(tb == 0), stop=(tb == nblk - 1),
                    )

            if (qb, 0) not in Stiles:
                Stiles[(qb, 0)] = emit_S(qb, 0)
            # O lags exp by 2 blocks (queue), so slow exps never stall
            # the in-order PE stream
            Pq = []
            for tb in range(nblk):
                psS = Stiles.pop((qb, tb))
                P = emit_exp(qb, tb, psS)
                if tb + 1 < nblk:
                    Stiles[(qb, tb + 1)] = emit_S(qb, tb + 1)
                elif qb + 1 < NT:
                    Stiles[(qb + 1, 0)] = emit_S(qb + 1, 0)
                for piece in emit_at.get(tb, ()):
                    emit_piece(piece)
                Pq.append((tb, P))
                if len(Pq) > 2:
                    otb, oP = Pq.pop(0)
                    emit_O(otb, oP)
            for otb, oP in Pq:
                emit_O(otb, oP)
            last = qb == NT - 1
            if last:
                # keep the PE clock warm through the final normalize; allocate
                # from ps_mm so the warmups don't contend for ps_q with the
                # in-flight late out-projection evacs
                for i in range(6):
                    wps = ps_mm.tile([128, HPC, 512], F32, tag="mm")
                    nc.tensor.matmul(wps[:, 0, :], warm_src[:, 0:128],
                                     warm_src[:], start=True, stop=True)
            # normalize: recip of the denominator row (psO[D]) out of PSUM
            # (staged via SBUF: the custom-DVE recip op does not read PSUM
            # correctly on HW), broadcast of the reciprocal on Pool, then one
            # mul into UnT
            rbrs = []
            for h in range(HPC):
                drow = small.tile([1, 512], F32, tag="drow")
                nc.vector.tensor_copy(drow[:], pos[h][D:D + 1, :])
                rbr = small.tile([1, 512], F32, tag="rbr")
                nc.vector.reciprocal_approx_fast(rbr[:], drow[:])
                rbrs.append(rbr)
            rbs = []
            for h in range(HPC):
                rb = small.tile([D, 512], F32, tag="rb")
                nc.gpsimd.partition_broadcast(rb[:], rbrs[h][:], channels=D)
                rbs.append(rb)
            if not last:
                for h in range(HPC):
                    nc.vector.tensor_mul(
                        UnT[h * D:(h + 1) * D, qb * 512:(qb + 1) * 512],
                        pos[h][0:D, :], rbs[h][:],
                    )
            else:
                # final block: normalize per 128-wide piece and launch each
                # out-projection tile as soon as its UnT columns settle
                for tl in range(4):
                    cs = slice(tl * 128, (tl + 1) * 128)
                    for h in range(HPC):
                        nc.vector.tensor_mul(
                            UnT[h * D:(h + 1) * D,
                                qb * 512 + tl * 128:qb * 512 + (tl + 1) * 128],
                            pos[h][0:D, cs], rbs[h][:, cs],
                        )
                    _outproj_tile(nc, ps_q, opool, UnT, wo_sb, out,
                                  (NT - 1) * 4 + tl, engs=("D", "A"))


def _outproj_tile(nc, ps_q, opool, UnT, wo_sb, out, tt, engs=("A", "A")):
    osb2 = opool.tile([128, E], BF16, tag="out")
    for half in range(2):
        psc = ps_q.tile([128, 512], F32, tag="q")
        nc.tensor.matmul(
            psc[:],
            UnT[:, tt * 128:(tt + 1) * 128],
            wo_sb[:, half * 512:(half + 1) * 512],
            start=True, stop=True,
        )
        dst = osb2[:, half * 512:(half + 1) * 512]
        if engs[half] == "A":
            nc.scalar.copy(dst, psc[:])
        else:
            nc.vector.tensor_copy(dst, psc[:])
    nc.sync.dma_start(out[tt * 128:(tt + 1) * 128, :], osb2[:])


_NC_CACHE = None


def _get_nc():
    global _NC_CACHE
    if _NC_CACHE is None:
        _NC_CACHE = _build_kernel()
    return _NC_CACHE


def _make_in_maps(x, w_qkv, b_qkv, w_out):
    x2 = np.asarray(x, dtype=np.float32).reshape(T, E)
    xT = np.ascontiguousarray(x2.T).astype(NPBF16)
    w_qkv = np.asarray(w_qkv, dtype=np.float32)
    b_qkv = np.asarray(b_qkv, dtype=np.float32)
    w_out = np.asarray(w_out, dtype=np.float32)
    in_maps = []
    for c in range(NCORES):
        s = slice(c * HD, (c + 1) * HD)
        in_maps.append({
            "xT": xT,
            "wq": np.ascontiguousarray(
                w_qkv[:, 0 * E + c * HD:0 * E + (c + 1) * HD]).astype(NPBF16),
            "wk": np.ascontiguousarray(
                w_qkv[:, 1 * E + c * HD:1 * E + (c + 1) * HD]).astype(NPBF16),
            "wv": np.ascontiguousarray(
                w_qkv[:, 2 * E + c * HD:2 * E + (c + 1) * HD]).astype(NPBF16),
            "bqk": np.ascontiguousarray(
                np.stack([
                    b_qkv[0 * E + c * HD:0 * E + (c + 1) * HD],
                    b_qkv[1 * E + c * HD:1 * E + (c + 1) * HD],
                ]).reshape(2, HD, 1)
            ),
            "wo": np.ascontiguousarray(w_out[s, :]).astype(NPBF16),
        })
    return in_maps


def run_sharded(x, w_qkv, b_qkv, w_out, b_out, trace=False):
    """Run the SPMD kernel; returns (full_output, BassKernelResults)."""
    nc = _get_nc()
    in_maps = _make_in_maps(x, w_qkv, b_qkv, w_out)
    res = run_bass_kernel_spmd(
        nc, in_maps, core_ids=list(range(NCORES)), trace=trace
    )
    acc = np.zeros((T, E), dtype=np.float32)
    for c in range(NCORES):
        acc += np.asarray(res.results[c]["out"], dtype=np.float32)
    b_qkv = np.asarray(b_qkv, dtype=np.float32)
    # v-bias shifts the attention output by b_v exactly -> fold on host
    acc += b_qkv[2 * E:3 * E] @ np.asarray(w_out, dtype=np.float32)
    acc += np.asarray(b_out, dtype=np.float32)[None, :]
    return acc.reshape(1, T, E), res


def kernel(x, w_qkv, b_qkv, w_out, b_out):
    out, _ = run_sharded(x, w_qkv, b_qkv, w_out, b_out, trace=False)
    return out


# revision 17
# speedup vs baseline: 1.0125x; 1.0125x over previous
"""Causal self-attention (B=1, T=4096, E=1024, H=16, D=64) on 8 TRN2 NeuronCores.

v2 of the head-parallel kernel. Same matmul structure as the baseline
(feature-major flash attention, 2 heads/core, row-tiled S^T pairs, ones-column
denominator trick), but the softmax exp — the baseline's pacer at ~150us of
ACT busy — is split across ACT and DVE:

  - ACT: exact exp (activation, scale=0.125) for ~43% of blocks
  - DVE: exp2 bit-trick for the rest: bits_i16 = S*23.083 + 16249, bitcast
    bf16 (Schraudolph). ~1.8% RMS multiplicative error, mean-centered; the
    denominator (ones column) uses the same approximated P, so the softmax
    ratio cancels most of it. Measured end-to-end rel err stays ~1e-2.

GpSimd (Pool) cannot touch PSUM on TRN2, so it only gets the SBUF-side work:
causal masks (affine_select on P) and the denominator-reciprocal broadcast.
ACT also absorbs the QKV and out-projection PSUM evacuations (Identity/Copy
live in the same activation table as Exp — no table reloads).

Other deltas vs baseline:
  - normalize: reciprocal of the denominator row straight out of PSUM,
    broadcast of the reciprocal, one mul -> UnT (posb/drow copies dropped)
  - v-bias never enters the kernel: host adds (b_v @ w_out) to the output
  - O' emission lags exp by 2 blocks so DVE-path exps never stall the PE
  - x chunk 0 DMA'd per-e-chunk so the first QKV matmul starts ~1.5us earlier
  - V' transposes batched 4-per-PSUM-tile, one evac copy per step
  - warmup matmuls trimmed
"""

import sys

for _p in ("/opt/trn_rl_repo",):
    if _p not in sys.path:
        sys.path.insert(0, _p)

import ml_dtypes
import numpy as np

import concourse.bass as bass  # noqa: F401
import concourse.mybir as mybir
import concourse.tile as tile
from concourse import bacc
from concourse.bass_utils import run_bass_kernel_spmd
from concourse.masks import make_identity

T, E = 4096, 1024
H, D = 16, 64
NCORES = 8
HPC = H // NCORES          # heads per core = 2
HD = HPC * D               # hidden dims per core = 128
NT = T // 512              # 8 tq blocks of 512
NE = E // 128              # 8 e-chunks of 128
NTB = T // 128             # 32 tk blocks of 128

F32 = mybir.dt.float32
BF16 = mybir.dt.bfloat16
I16 = mybir.dt.int16
NPBF16 = np.dtype(ml_dtypes.bfloat16)
AF = mybir.ActivationFunctionType
ALU = mybir.AluOpType

# exp2 bit trick: exp(0.125*S) = 2^(0.125*log2(e)*S);
# bf16 bits = x*128 + 127*128, -7 centers the linear-interp error
EXP_A = 128.0 * 0.125 * 1.4426950408889634
EXP_B = 16256.0 - 7.0

# exp engine schedule (A=ACT exact, D=DVE bit trick). Early q-blocks have
# plenty of PE filler work (QKV/outproj) stretching the block period, so ACT
# can take a bigger share; late q-blocks run close to the raw S/O cadence and
# need the two engines balanced by their speed ratio.
FULL_PAT_EARLY = "ADDADAD"
FULL_PAT_LATE = "AADAD"
DIAG_PAT = "AD"
# out-projection tiles per pipeline step (skewed late: early steps already
# have QKV+vtrans fillers; late steps otherwise run out of PE filler and
# stall on exp)
OPROJ_QUOTA = {2: 2, 3: 4, 4: 4, 5: 6, 6: 8, 7: 4}


def _build_kernel():
    nc = bacc.Bacc("TRN2", target_bir_lowering=False, debug=False)

    xT = nc.dram_tensor("xT", [E, T], BF16, kind="ExternalInput")
    wq = nc.dram_tensor("wq", [E, HD], BF16, kind="ExternalInput")
    wk = nc.dram_tensor("wk", [E, HD], BF16, kind="ExternalInput")
    wv = nc.dram_tensor("wv", [E, HD], BF16, kind="ExternalInput")
    bqk = nc.dram_tensor("bqk", [2, HD, 1], F32, kind="ExternalInput")
    wo = nc.dram_tensor("wo", [HD, E], BF16, kind="ExternalInput")
    out = nc.dram_tensor("out", [T, E], BF16, kind="ExternalOutput")

    with tile.TileContext(nc) as tc:
        _body(nc, tc, xT, wq, wk, wv, bqk, wo, out)
    nc.compile()
    return nc


def _body(nc, tc, xT, wq, wk, wv, bqk, wo, out):
    from contextlib import ExitStack

    ctx = ExitStack()
    with ctx:
        const = ctx.enter_context(tc.tile_pool(name="const", bufs=1))
        big = ctx.enter_context(tc.tile_pool(name="big", bufs=1))
        xpool = ctx.enter_context(tc.tile_pool(name="xp", bufs=3))
        ppool = ctx.enter_context(tc.tile_pool(name="pp", bufs=4))
        opool = ctx.enter_context(tc.tile_pool(name="op", bufs=3))
        small = ctx.enter_context(tc.tile_pool(name="sm", bufs=4))
        ps_mm = ctx.enter_context(tc.tile_pool(name="ps_mm", bufs=2, space="PSUM"))
        ps_o = ctx.enter_context(tc.tile_pool(name="ps_o", bufs=2, space="PSUM"))
        ps_q = ctx.enter_context(tc.tile_pool(name="ps_q", bufs=2, space="PSUM"))

        # ---- constants / weights ----
        warm_src = const.tile([128, 512], BF16)
        nc.vector.memset(warm_src[:], 0.0)
        identb = const.tile([128, 128], BF16)
        make_identity(nc, identb[:])

        xs_map = {}

        def load_x(tcc, interleave=None):
            ts512 = slice(tcc * 512, (tcc + 1) * 512)
            xsb = xpool.tile([128, NE, 512], BF16, tag="xsb")
            if interleave is not None:
                # per-e-chunk DMAs so the first QKV matmuls start after
                # ~1/8 of the chunk has landed; weight DMAs interleaved
                for ec in range(NE):
                    nc.sync.dma_start(
                        xsb[:, ec, :], xT[ec * 128:(ec + 1) * 128, ts512]
                    )
                    for fn in dict(interleave).get(ec, ()):
                        fn()
            else:
                nc.sync.dma_start(
                    xsb[:], xT[:, ts512].rearrange("(a p) t -> p a t", p=128)
                )
            xs_map[tcc] = xsb

        wq_sb = const.tile([128, NE, HD], BF16)
        wk_sb = const.tile([128, NE, HD], BF16)
        wv_sb = const.tile([128, NE, HD], BF16)
        bq_sb = const.tile([128, 1], F32)
        bk_sb = const.tile([128, 1], F32)

        # weights ride the (otherwise idle) GpSimd DMA queue, concurrent
        # with the x chunks on the sync queue
        nc.gpsimd.dma_start(wq_sb[:], wq[:].rearrange("(a p) c -> p a c", p=128))
        nc.gpsimd.dma_start(bq_sb[:], bqk[0])
        nc.gpsimd.dma_start(bk_sb[:], bqk[1])
        nc.gpsimd.dma_start(wk_sb[:], wk[:].rearrange("(a p) c -> p a c", p=128))
        load_x(0, interleave={})
        nc.gpsimd.dma_start(wv_sb[:], wv[:].rearrange("(a p) c -> p a c", p=128))
        load_x(1)
        wo_sb = const.tile([128, E], BF16)
        nc.gpsimd.dma_start(wo_sb[:], wo[:])

        qT = big.tile([128, T], BF16)
        kT = big.tile([128, T], BF16)
        vT = big.tile([128, T], BF16)
        V2 = big.tile([128, NTB, HPC, D + 1], BF16)
        UnT = big.tile([128, T], BF16)

        nc.gpsimd.memset(V2[:, :, :, D], 1.0)

        wparams = ((wq_sb, bq_sb), (wk_sb, bk_sb), (wv_sb, None))
        qkv_dst = (qT, kT, vT)

        def emit_qkv(tcc, m):
            w_sb, b_sb = wparams[m]
            ts512 = slice(tcc * 512, (tcc + 1) * 512)
            ps = ps_q.tile([128, 512], F32, tag="q")
            for ec in range(NE):
                nc.tensor.matmul(
                    ps[:], w_sb[:, ec, :], xs_map[tcc][:, ec, :],
                    start=(ec == 0), stop=(ec == NE - 1),
                )
            # evac on ACT (same act table as Exp): q/k get the bias,
            # v's bias is folded host-side
            if b_sb is not None:
                nc.scalar.activation(
                    qkv_dst[m][:, ts512], ps[:], AF.Identity,
                    bias=b_sb[:], scale=1.0,
                )
            else:
                nc.scalar.copy(qkv_dst[m][:, ts512], ps[:])

        def emit_vtrans(tcc):
            # all 4 tk blocks of the chunk into one PSUM tile, one evac.
            # start only on j==0: a start=True matmul pending-zeroes its whole
            # 2KB PSUM bank (ZERO_REGION_SIZE), which would wipe earlier
            # transposes sharing the tile.
            pst = ps_q.tile([128, 4, 128], BF16, tag="q")
            for j in range(4):
                tb = 4 * tcc + j
                nc.tensor.matmul(
                    pst[:, j, :], vT[:, tb * 128:(tb + 1) * 128], identb[:],
                    is_transpose=True, start=(j == 0), stop=(j == 3),
                )
            nc.vector.tensor_copy(
                V2[:, 4 * tcc:4 * tcc + 4, :, 0:D],
                pst[:].rearrange("p j (h d) -> p j h d", h=HPC),
            )

        pending_norm = [None]

        def make_norm(qb_, pos_, split_=False):
            """Normalize closure: recip of the denominator row (staged via
            SBUF — the custom-DVE recip op does not read PSUM correctly on
            HW), broadcast of the reciprocal on Pool, muls into UnT. Deferred
            into the NEXT step so the chain never delays boundary exps on
            the DVE queue."""
            def _emit():
                rbrs = []
                for h in range(HPC):
                    drow = small.tile([1, 512], F32, tag="drow")
                    nc.vector.tensor_copy(drow[:], pos_[h][D:D + 1, :])
                    rbr = small.tile([1, 512], F32, tag="rbr")
                    nc.vector.reciprocal_approx_fast(rbr[:], drow[:])
                    rbrs.append(rbr)
                rbs = []
                for h in range(HPC):
                    rb = small.tile([D, 512], F32, tag="rb")
                    nc.gpsimd.partition_broadcast(rb[:], rbrs[h][:], channels=D)
                    rbs.append(rb)
                if not split_:
                    for h in range(HPC):
                        nc.vector.tensor_mul(
                            UnT[h * D:(h + 1) * D, qb_ * 512:(qb_ + 1) * 512],
                            pos_[h][0:D, :], rbs[h][:],
                        )
                else:
                    # final block: normalize per 128-wide piece and launch
                    # each out-projection tile as soon as its columns settle
                    for tl in range(4):
                        cs = slice(tl * 128, (tl + 1) * 128)
                        for h in range(HPC):
                            nc.vector.tensor_mul(
                                UnT[h * D:(h + 1) * D,
                                    qb_ * 512 + tl * 128:
                                    qb_ * 512 + (tl + 1) * 128],
                                pos_[h][0:D, cs], rbs[h][:, cs],
                            )
                        _outproj_tile(nc, ps_q, opool, UnT, wo_sb, out,
                                      (NT - 1) * 4 + tl, engs=("D", "A"))
            return _emit

        def emit_piece(piece):
            kind = piece[0]
            if kind == "qkv":
                emit_qkv(piece[1], piece[2])
            elif kind == "vtrans":
                emit_vtrans(piece[1])
            elif kind == "norm":
                fn, pending_norm[0] = pending_norm[0], None
                fn()
            else:
                _outproj_tile(nc, ps_q, opool, UnT, wo_sb, out, piece[1],
                              engs=("D", "A"))

        def emit_S(qb, tb):
            f0 = max(0, tb * 128 - qb * 512)
            psS = ps_mm.tile([128, HPC, 512], F32, tag="mm")
            t0 = qb * 512 + f0
            t1 = (qb + 1) * 512
            for h in range(HPC):
                nc.tensor.matmul(
                    psS[:, h, f0:512],
                    kT[h * D:(h + 1) * D, tb * 128:(tb + 1) * 128],
                    qT[h * D:(h + 1) * D, t0:t1],
                    start=True, stop=True,
                )
            return psS

        exp_ct = [0, 0]  # full counter, diag counter

        def emit_exp(qb, tb, psS):
            """exp(0.125*S) -> P bf16; engine from schedule; returns P."""
            diag = tb >= 4 * qb
            f0 = max(0, tb * 128 - qb * 512)
            P = ppool.tile([128, HPC, 512], BF16, tag="P")
            if diag:
                eng = DIAG_PAT[exp_ct[1] % len(DIAG_PAT)]
                exp_ct[1] += 1
            else:
                pat = FULL_PAT_EARLY if qb < 5 else FULL_PAT_LATE
                eng = pat[exp_ct[0] % len(pat)]
                exp_ct[0] += 1
            if eng == "A":
                if f0 == 0:
                    nc.scalar.activation(
                        P[:].rearrange("p h f -> p (h f)"),
                        psS[:].rearrange("p h f -> p (h f)"),
                        AF.Exp, scale=0.125,
                    )
                else:
                    nc.scalar.activation(
                        P[:, :, f0:512], psS[:, :, f0:512], AF.Exp, scale=0.125
                    )
            else:
                if f0 == 0:
                    nc.vector.tensor_scalar(
                        P[:].rearrange("p h f -> p (h f)").bitcast(I16),
                        psS[:].rearrange("p h f -> p (h f)"),
                        EXP_A, EXP_B, ALU.mult, ALU.add,
                    )
                else:
                    nc.vector.tensor_scalar(
                        P[:, :, f0:512].bitcast(I16),
                        psS[:, :, f0:512],
                        EXP_A, EXP_B, ALU.mult, ALU.add,
                    )
            if diag:
                # causal mask on Pool: keep where tq >= tk
                nc.gpsimd.affine_select(
                    out=P[:, :, f0:512], in_=P[:, :, f0:512],
                    compare_op=mybir.AluOpType.is_ge,
                    fill=0.0,
                    base=qb * 512 + f0 - tb * 128,
                    channel_multiplier=-1,
                    pattern=[[0, HPC], [1, 512 - f0]],
                )
            return P

        # ---- prologue: short warmups span the x0/wq DMA wait and ramp
        # the PE clock; then chunk-0 q/k projections gate the first S ----
        for i in range(12):
            wps = ps_q.tile([128, 128], F32, tag="q")
            nc.tensor.matmul(wps[:], warm_src[:, 0:128], warm_src[:, 0:128],
                             start=True, stop=True)
        emit_qkv(0, 0)
        emit_qkv(0, 1)

        # ---- merged pipeline ----
        Stiles = {}
        oproj_next = [0]
        for step in range(NT):
            if step + 2 < NT:
                load_x(step + 2)
            pieces = []
            if pending_norm[0] is not None:
                pieces += [("norm",)]
            if step == 0:
                pieces += [("qkv", 0, 2)]
                pieces += [("vtrans", 0)]
            if step + 1 < NT:
                pieces += [("qkv", step + 1, m) for m in range(3)]
                pieces += [("vtrans", step + 1)]
            quota = OPROJ_QUOTA.get(step, 0)
            late = []
            if step == NT - 1:
                late += [("out", oproj_next[0] + i) for i in range(quota)]
            else:
                pieces += [("out", oproj_next[0] + i) for i in range(quota)]
            oproj_next[0] += quota
            qb = step
            nblk = 4 * (qb + 1)
            emit_at = {}
            for i, piece in enumerate(pieces):
                emit_at.setdefault((i + 1) * nblk // (len(pieces) + 1),
                                   []).append(piece)
            lo = nblk // 2
            for i, piece in enumerate(late):
                emit_at.setdefault(lo + (i + 1) * (nblk - lo) // (len(late) + 1),
                                   []).append(piece)
            pos = []
            for h in range(HPC):
                po = ps_o.tile([D + 1, 512], F32, tag="o")
                pos.append(po)

            def emit_O(tb, P):
                f0 = max(0, tb * 128 - qb * 512)
                for h in range(HPC):
                    nc.tensor.matmul(
                        pos[h][:, f0:512],
                        V2[:, tb, h, :],
                        P[:, h, f0:512],
                        start=(tb == 0), stop=(tb == nblk - 1),
                    )

            if (qb, 0) not in Stiles:
                Stiles[(qb, 0)] = emit_S(qb, 0)
            # O lags exp by 2 blocks (queue), so slow exps never stall
            # the in-order PE stream
            Pq = []
            for tb in range(nblk):
                psS = Stiles.pop((qb, tb))
                P = emit_exp(qb, tb, psS)
                if tb + 1 < nblk:
                    Stiles[(qb, tb + 1)] = emit_S(qb, tb + 1)
                elif qb + 1 < NT:
                    Stiles[(qb + 1, 0)] = emit_S(qb + 1, 0)
                for piece in emit_at.get(tb, ()):
                    emit_piece(piece)
                Pq.append((tb, P))
                if len(Pq) > 2:
                    otb, oP = Pq.pop(0)
                    emit_O(otb, oP)
            for otb, oP in Pq:
                emit_O(otb, oP)
            if qb < NT - 1:
                pending_norm[0] = make_norm(qb, pos)
            else:
                # keep the PE clock warm through the final normalize; allocate
                # from ps_mm so the warmups don't contend for ps_q with the
                # in-flight late out-projection evacs
                for i in range(12):
                    wps = ps_mm.tile([128, HPC, 512], F32, tag="mm")
                    nc.tensor.matmul(wps[:, 0, :], warm_src[:, 0:128],
                                     warm_src[:], start=True, stop=True)
                make_norm(qb, pos, split_=True)()


def _outproj_tile(nc, ps_q, opool, UnT, wo_sb, out, tt, engs=("A", "A")):
    osb2 = opool.tile([128, E], BF16, tag="out")
    for half in range(2):
        psc = ps_q.tile([128, 512], F32, tag="q")
        nc.tensor.matmul(
            psc[:],
            UnT[:, tt * 128:(tt + 1) * 128],
            wo_sb[:, half * 512:(half + 1) * 512],
            start=True, stop=True,
        )
        dst = osb2[:, half * 512:(half + 1) * 512]
        if engs[half] == "A":
            nc.scalar.copy(dst, psc[:])
        else:
            nc.vector.tensor_copy(dst, psc[:])
    nc.sync.dma_start(out[tt * 128:(tt + 1) * 128, :], osb2[:])


_NC_CACHE = None


def _get_nc():
    global _NC_CACHE
    if _NC_CACHE is None:
        _NC_CACHE = _build_kernel()
    return _NC_CACHE


def _make_in_maps(x, w_qkv, b_qkv, w_out):
    x2 = np.asarray(x, dtype=np.float32).reshape(T, E)
    xT = np.ascontiguousarray(x2.T).astype(NPBF16)
    w_qkv = np.asarray(w_qkv, dtype=np.float32)
    b_qkv = np.asarray(b_qkv, dtype=np.float32)
    w_out = np.asarray(w_out, dtype=np.float32)
    in_maps = []
    for c in range(NCORES):
        s = slice(c * HD, (c + 1) * HD)
        in_maps.append({
            "xT": xT,
            "wq": np.ascontiguousarray(
                w_qkv[:, 0 * E + c * HD:0 * E + (c + 1) * HD]).astype(NPBF16),
            "wk": np.ascontiguousarray(
                w_qkv[:, 1 * E + c * HD:1 * E + (c + 1) * HD]).astype(NPBF16),
            "wv": np.ascontiguousarray(
                w_qkv[:, 2 * E + c * HD:2 * E + (c + 1) * HD]).astype(NPBF16),
            "bqk": np.ascontiguousarray(
                np.stack([
                    b_qkv[0 * E + c * HD:0 * E + (c + 1) * HD],
                    b_qkv[1 * E + c * HD:1 * E + (c + 1) * HD],
                ]).reshape(2, HD, 1)
            ),
            "wo": np.ascontiguousarray(w_out[s, :]).astype(NPBF16),
        })
    return in_maps


def run_sharded(x, w_qkv, b_qkv, w_out, b_out, trace=False):
    """Run the SPMD kernel; returns (full_output, BassKernelResults)."""
    nc = _get_nc()
    in_maps = _make_in_maps(x, w_qkv, b_qkv, w_out)
    res = run_bass_kernel_spmd(
        nc, in_maps, core_ids=list(range(NCORES)), trace=trace
    )
    acc = np.zeros((T, E), dtype=np.float32)
    for c in range(NCORES):
        acc += np.asarray(res.results[c]["out"], dtype=np.float32)
    b_qkv = np.asarray(b_qkv, dtype=np.float32)
    # v-bias shifts the attention output by b_v exactly -> fold on host
    acc += b_qkv[2 * E:3 * E] @ np.asarray(w_out, dtype=np.float32)
    acc += np.asarray(b_out, dtype=np.float32)[None, :]
    return acc.reshape(1, T, E), res


def kernel(x, w_qkv, b_qkv, w_out, b_out):
    out, _ = run_sharded(x, w_qkv, b_qkv, w_out, b_out, trace=False)
    return out
